# revision 40
# baseline (speedup 1.0000x reference)
"""Multi-head attention (b=2, sq=skv=2048, dim=1024, 16 heads x 64) on 8 TRN2
NeuronCores.

Sharding: 2 heads per core (head-parallel across batch*heads), with the
matching tensor-parallel column slice of W_qkv and row slice of W_out.  Each
core computes a partial output projection over its 128 head-dims; the
all-reduce of the 8 partials (+ bias) happens on the host during unshard.

Per-core kernel (bf16 compute, fp32 PSUM accumulation):
  phase 1: qT/kT/vT = W.T @ x.T   ([128 = 2 heads x 64 dims, tokens]); v is
           additionally PE-transposed to natural [token, dim] layout with a
           ones column appended (denominator trick).
  phase 2: per (batch, q-tile, k-tile): scoresT for both heads ([k-tokens, q])
           in one 2-bank PSUM group; one exp ACTIVATE over the group (scale
           1/8 fused, no max subtraction -- scores range +-10); PV matmuls
           accumulate [v | 1].T @ expT over the 16 k-tiles giving unnormalized
           outT plus the softmax denominator in row 64.  The accumulator is
           copied to SBUF immediately (releasing PSUM); normalization
           (reciprocal + PE outer-product broadcast + multiply) happens off
           the critical path.
  phase 3: partial out = outT.T @ W_out_rows -> bf16 [tokens, 1024].

Emission is orchestrated so the dependency-driven Tile scheduler always has
filler PE work (batch-1 projections, out-projection quarters) inside the
ACT(exp)-bound attention stream, keeping the PE HAM-warm.
"""

import os
import sys

for _p in ("/opt/trn_rl_repo", "/root/.axon_site/_ro/trn_rl_repo"):
    if os.path.isdir(_p) and _p not in sys.path:
        sys.path.append(_p)

import ml_dtypes
import numpy as np

import concourse.bass as bass  # noqa: F401
import concourse.tile as tile
from concourse import bacc, mybir
from concourse.bass_utils import run_bass_kernel_spmd
from concourse.masks import make_identity

B, SQ, SKV, DIM = 2, 2048, 2048, 1024
HEADS, DH = 16, 64
N_CORES = 8
HPC = HEADS // N_CORES  # heads per core = 2
HD = HPC * DH  # 128 head-dim rows per core
TOK = B * SQ  # 4096
KO = DIM // 128  # 8 contraction chunks of 128
SCALE = DH**-0.5

BF16 = mybir.dt.bfloat16
F32 = mybir.dt.float32

PCHUNK = 512  # token chunk in projections (contiguous per-chunk dram layout)
QTILE = 512  # q tile in attention
KTILE = 128  # k tile (scores psum partition dim)
NKT = SKV // KTILE  # 16
NQT = SQ // QTILE  # 4

BF = ml_dtypes.bfloat16
Exp = mybir.ActivationFunctionType.Exp


def build():
    nc = bacc.Bacc(
        "TRN2", target_bir_lowering=False, debug=False, num_devices=N_CORES
    )

    NCH = TOK // PCHUNK
    xqt_d = nc.dram_tensor("xqt", [NCH, 128, KO, PCHUNK], BF16, kind="ExternalInput")
    xkvt_d = nc.dram_tensor("xkvt", [NCH, 128, KO, PCHUNK], BF16, kind="ExternalInput")
    wq_d = nc.dram_tensor("wq", [DIM, HD], BF16, kind="ExternalInput")
    wk_d = nc.dram_tensor("wk", [DIM, HD], BF16, kind="ExternalInput")
    wv_d = nc.dram_tensor("wv", [DIM, HD], BF16, kind="ExternalInput")
    wout_d = nc.dram_tensor("wout", [HD, DIM], BF16, kind="ExternalInput")
    out_d = nc.dram_tensor("out", [TOK, DIM], BF16, kind="ExternalOutput")

    xqt = xqt_d.ap()
    xkvt = xkvt_d.ap()

    with tile.TileContext(nc) as tc:
        with (
            tc.tile_pool(name="persist", bufs=1) as persist,
            tc.tile_pool(name="xin", bufs=6) as xin,
            tc.tile_pool(name="exps", bufs=10) as exps,
            tc.tile_pool(name="ost", bufs=3) as ost,
            tc.tile_pool(name="spsum", bufs=2, space="PSUM") as spsum,
            tc.tile_pool(name="accp", bufs=2, space="PSUM") as accp,
            tc.tile_pool(name="miscp", bufs=2, space="PSUM") as miscp,
            tc.tile_pool(name="drp", bufs=2, space="DRAM") as drp,
        ):
            # --- weights / constants ---
            wq_sb = persist.tile([128, KO, HD], BF16, tag="wq")
            nc.gpsimd.dma_start(wq_sb[:], wq_d.ap().rearrange("(ko p) m -> p ko m", p=128))
            wk_sb = persist.tile([128, KO, HD], BF16, tag="wk")
            nc.gpsimd.dma_start(wk_sb[:], wk_d.ap().rearrange("(ko p) m -> p ko m", p=128))
            wv_sb = persist.tile([128, KO, HD], BF16, tag="wv")
            nc.gpsimd.dma_start(wv_sb[:], wv_d.ap().rearrange("(ko p) m -> p ko m", p=128))
            wout_sb = persist.tile([HD, DIM], BF16, tag="wout")
            nc.gpsimd.dma_start(wout_sb[:], wout_d.ap())

            ident = persist.tile([128, DH], BF16, tag="ident")
            make_identity(nc, ident[0:DH, :])
            make_identity(nc, ident[DH : 2 * DH, :])
            ones_f32 = persist.tile([1, DH], F32, tag="ones")
            nc.vector.memset(ones_f32[:], 1.0)
            # prefetch the exp table set during the head DMAs
            dummy = persist.tile([1, 8], F32, tag="dummy")
            nc.vector.memset(dummy[:], 0.0)
            nc.scalar.activation(dummy[:], dummy[:], Exp)

            qt_sb, kt_sb, vt_sb, vnat, outT, usb = {}, {}, {}, {}, {}, {}
            for b in range(B):
                qt_sb[b] = persist.tile([HD, SQ], BF16, tag=f"qt{b}", name=f"qt{b}")
                kt_sb[b] = persist.tile([HD, SKV], BF16, tag=f"kt{b}", name=f"kt{b}")
                vt_sb[b] = persist.tile([HD, SKV], BF16, tag=f"vt{b}", name=f"vt{b}")
                vnat[b] = persist.tile(
                    [128, HPC, NKT, DH + 1], BF16, tag=f"vn{b}", name=f"vn{b}"
                )
                outT[b] = persist.tile([HD, SQ], BF16, tag=f"ot{b}", name=f"ot{b}")
                # unnormalized outT + denominators, unit index = qt*HPC + h
                usb[b] = persist.tile(
                    [DH + 1, NQT * HPC, QTILE], F32, tag=f"us{b}", name=f"us{b}"
                )
                nc.vector.memset(vnat[b][:, :, :, DH], 1.0)

            def _proj(dst, w_sb, xt, tt):
                for sub in range(PCHUNK // 512):
                    _proj_sub(dst, w_sb, xt, tt, sub)

            def load_chunk(x_ap, tok0, tt, eng=None):
                xt = xin.tile([128, KO, PCHUNK], BF16, tag="x")
                (eng or nc.sync).dma_start(
                    xt[:], x_ap[(tok0 + tt * PCHUNK) // PCHUNK]
                )
                return xt

            def _proj_sub(dst, w_sb, xt, tt, sub):
                ps = miscp.tile([128, 512], F32, tag="m", name="projp")
                for ko in range(KO):
                    nc.tensor.matmul(
                        ps[:],
                        w_sb[:, ko, :],
                        xt[:, ko, sub * 512 : (sub + 1) * 512],
                        start=(ko == 0),
                        stop=(ko == KO - 1),
                    )
                t0 = tt * PCHUNK + sub * 512
                nc.vector.tensor_copy(dst[:, t0 : t0 + 512], ps[:])

            def vnat_group(b, jg):
                """PE-transpose k-tiles 4jg..4jg+3 of vT into natural layout."""
                for h in range(HPC):
                    tp = miscp.tile([128, 4, DH], BF16, tag="m", name="vtp")
                    for i in range(4):
                        j = jg * 4 + i
                        nc.tensor.transpose(
                            tp[:, i, :],
                            vt_sb[b][
                                h * DH : (h + 1) * DH,
                                j * KTILE : (j + 1) * KTILE,
                            ],
                            ident[h * DH : (h + 1) * DH, :],
                        )
                    nc.vector.tensor_copy(
                        vnat[b][:, h, jg * 4 : (jg + 1) * 4, 0:DH], tp[:]
                    )

            F32R = mybir.dt.float32r

            def norm_flush(b, u0, nu):
                """Normalize units u0..u0+nu-1 of usb[b] into outT[b].

                Batches the reciprocal: denominator rows are bounced through
                DRAM to repack [1, nu, QTILE] -> [128, nu*QTILE/128] so the
                DVE reciprocal runs wide, then bounced back and broadcast to
                64 partitions with an f32r PE outer product.
                """
                nel = nu * QTILE
                d1 = drp.tile([1, nu, QTILE], F32, tag="d1", name="d1")
                nc.gpsimd.dma_start(d1[:], usb[b][DH : DH + 1, u0 : u0 + nu, :])
                dpk = ost.tile([128, nel // 128], F32, tag="dp", name="dpk")
                nc.gpsimd.dma_start(
                    dpk[:],
                    d1[:]
                    .rearrange("a b c -> (a b c)")
                    .rearrange("(p f) -> p f", p=128),
                )
                rpk = ost.tile([128, nel // 128], F32, tag="rp", name="rpk")
                nc.vector.reciprocal(rpk[:], dpk[:])
                d2 = drp.tile([1, nu, QTILE], F32, tag="d2", name="d2")
                nc.gpsimd.dma_start(
                    d2[:]
                    .rearrange("a b c -> (a b c)")
                    .rearrange("(p f) -> p f", p=128),
                    rpk[:],
                )
                rst = ost.tile([1, nu, QTILE], F32, tag="rs", name="rst")
                nc.gpsimd.dma_start(rst[:], d2[:])
                for i in range(nu):
                    g = u0 + i
                    qt, h = divmod(g, HPC)
                    bc = miscp.tile([DH, QTILE], F32, tag="m", name="bc")
                    nc.tensor.matmul(
                        bc[:],
                        ones_f32[:].bitcast(F32R),
                        rst[0:1, i, :].bitcast(F32R),
                        start=True,
                        stop=True,
                    )
                    nc.vector.tensor_mul(
                        outT[b][h * DH : (h + 1) * DH, qt * QTILE : (qt + 1) * QTILE],
                        usb[b][0:DH, g, :],
                        bc[:],
                    )

            LOOKAHEAD = 6

            def attention(b, hooks):
                """Flat software-pipelined attention over all (qt, j) steps.

                Scores for step t+2 are emitted before PV of step t, so the
                PE always has score matmuls queued ahead of the exp/PV chain
                and q-tile boundaries pipeline seamlessly.  hooks[qt] is a
                dict keyed (qt, j) of emission callables fired right
                after that step's PV matmuls, spreading filler work finely
                through the ACT-bound stream.
                """
                NT = NQT * NKT
                sps, accs = {}, {}

                def emit_scores(t):
                    qt, j = divmod(t, NKT)
                    q_sl = slice(qt * QTILE, (qt + 1) * QTILE)
                    k_sl = slice(j * KTILE, (j + 1) * KTILE)
                    sp = spsum.tile([128, HPC, QTILE], F32, tag="s", name="sp")
                    sps[t] = sp
                    for h in range(HPC):
                        h_sl = slice(h * DH, (h + 1) * DH)
                        nc.tensor.matmul(
                            sp[:, h, :],
                            kt_sb[b][h_sl, k_sl],
                            qt_sb[b][h_sl, q_sl],
                            start=True,
                            stop=True,
                        )

                def emit_tail(t):
                    qt, j = divmod(t, NKT)
                    sp = sps.pop(t)
                    ex = exps.tile([128, HPC, QTILE], BF16, tag="e", name="ex")
                    nc.scalar.activation(ex[:], sp[:], Exp, scale=SCALE)
                    if j == 0:
                        accs[qt] = [
                            accp.tile([128, QTILE], F32, tag="acc", name="acc")
                            for _ in range(HPC)
                        ]
                    for h in range(HPC):
                        nc.tensor.matmul(
                            accs[qt][h][0 : DH + 1, :],
                            vnat[b][:, h, j, :],
                            ex[:, h, :],
                            start=(j == 0),
                            stop=(j == NKT - 1),
                        )
                    if j == NKT - 1:
                        for h in range(HPC):
                            # free the PSUM accumulator; normalization comes
                            # later in norm_flush
                            nc.vector.tensor_copy(
                                usb[b][:, qt * HPC + h, :],
                                accs[qt][h][0 : DH + 1, :],
                            )
                        del accs[qt]
                    for fn in hooks.get((qt, j), []):
                        fn()

                for t in range(NT + LOOKAHEAD):
                    if t < NT:
                        emit_scores(t)
                    if t >= LOOKAHEAD:
                        emit_tail(t - LOOKAHEAD)

            def outproj(b, tt0, tt1, split_copy=False):
                for tt in range(tt0, tt1):
                    t_sl = slice(tt * 128, (tt + 1) * 128)
                    ob = ost.tile([128, 2, 512], BF16, tag="o")
                    for nt in range(DIM // 512):
                        ps = miscp.tile([128, 512], F32, tag="m", name="projo")
                        nc.tensor.matmul(
                            ps[:],
                            outT[b][:, t_sl],
                            wout_sb[:, nt * 512 : (nt + 1) * 512],
                            start=True,
                            stop=True,
                        )
                        if split_copy and nt % 2 == 0:
                            nc.scalar.copy(ob[:, nt, :], ps[:])
                        else:
                            nc.vector.tensor_copy(ob[:, nt, :], ps[:])
                    nc.gpsimd.dma_start(
                        out_d.ap()[
                            b * SQ + tt * 128 : b * SQ + (tt + 1) * 128, :
                        ].rearrange("t (n c) -> t n c", n=2),
                        ob[:],
                    )

            def qkv_pieces(b):
                """Projection emission steps, 512-token chunks.  K chunks
                first (scores consume them progressively); each x_kv chunk
                is loaded once for both K and V; Q tiles beyond the first
                q-tile come last."""
                xts = {}

                def kv_load_k(tt):
                    xts[tt] = load_chunk(xkvt, b * SKV, tt)
                    _proj(kt_sb[b], wk_sb, xts[tt], tt)

                def v_part(tt):
                    _proj(vt_sb[b], wv_sb, xts.pop(tt), tt)
                    vnat_group(b, tt)

                yield lambda: kv_load_k(0)
                yield lambda: proj_chunk(qt_sb[b], wq_sb, xqt, b * SQ, 0)
                for tt in range(1, SQ // PCHUNK):
                    yield lambda tt=tt: kv_load_k(tt)
                for tt in range(SQ // PCHUNK):
                    yield lambda tt=tt: v_part(tt)
                for tt in range(1, SQ // PCHUNK):
                    yield lambda tt=tt: proj_chunk(qt_sb[b], wq_sb, xqt, b * SQ, tt)

            def proj_chunk(dst, w_sb, x_ap, tok0, tt):
                _proj(dst, w_sb, load_chunk(x_ap, tok0, tt), tt)

            # --- emission schedule: fine-grained interleave so the scheduler
            # always has dep-free PE work to fill ACT-bound attention gaps ---
            for piece in qkv_pieces(0):
                piece()

            nxt = qkv_pieces(1)

            def emit_next():
                p = next(nxt, None)
                if p is not None:
                    p()

            def emit_n(n):
                def go():
                    for _ in range(n):
                        emit_next()

                return go

            def flush_op(b, qt):
                def go():
                    norm_flush(b, qt * HPC, HPC)
                    outproj(b, qt * 4, qt * 4 + 4)

                return go

            hooks0 = {
                (0, 15): [emit_n(4)],
                (1, 15): [emit_n(4), flush_op(0, 0)],
                (2, 15): [emit_n(4), flush_op(0, 1)],
                (3, 15): [flush_op(0, 2)],
            }
            attention(0, hooks0)

            hooks1 = {
                (0, 0): [lambda: norm_flush(0, 3 * HPC, HPC)],
                (0, 3): [emit_next],
                (0, 8): [lambda: outproj(0, 12, 16), emit_next],
                (0, 13): [emit_next],
                (1, 15): [flush_op(1, 0)],
                (2, 15): [flush_op(1, 1)],
                (3, 15): [flush_op(1, 2)],
            }
            attention(1, hooks1)
            norm_flush(1, 3 * HPC, HPC)
            outproj(1, 12, 16, split_copy=True)

    nc.compile()
    return nc


def make_in_maps(x_q, x_kv, W_qkv, W_out):
    x_q = np.asarray(x_q, dtype=np.float32)
    x_kv = np.asarray(x_kv, dtype=np.float32)
    W_qkv = np.asarray(W_qkv, dtype=np.float32)
    W_out = np.asarray(W_out, dtype=np.float32)

    def chunk_tile(x):
        # [TOK, DIM] -> [n_chunks, 128, KO, PCHUNK] with D = ko*128 + p
        xt = x.reshape(TOK, DIM).T.reshape(KO, 128, TOK // PCHUNK, PCHUNK)
        return np.ascontiguousarray(xt.transpose(2, 1, 0, 3)).astype(BF)

    xqt = chunk_tile(x_q)
    xkvt = chunk_tile(x_kv)

    in_maps = []
    for c in range(N_CORES):
        cs = slice(c * HD, (c + 1) * HD)
        in_maps.append(
            {
                "xqt": xqt,
                "xkvt": xkvt,
                "wq": np.ascontiguousarray(W_qkv[:, cs]).astype(BF),
                "wk": np.ascontiguousarray(W_qkv[:, 1024:][:, cs]).astype(BF),
                "wv": np.ascontiguousarray(W_qkv[:, 2048:][:, cs]).astype(BF),
                "wout": np.ascontiguousarray(W_out[cs, :]).astype(BF),
            }
        )
    return in_maps


def combine(partials, b_out):
    """Sum the 8 per-core partial projections and add the bias."""
    acc = np.zeros((TOK, DIM), dtype=np.float32)
    for p in partials:
        acc += np.asarray(p, dtype=np.float32)
    acc += np.asarray(b_out, dtype=np.float32)
    return acc.reshape(B, SQ, DIM)


_STATE = {}


def _get_nc():
    if "nc" not in _STATE:
        _STATE["nc"] = build()
    return _STATE["nc"]


def run(x_q, x_kv, W_qkv, W_out, b_out, trace=False):
    nc = _get_nc()
    in_maps = make_in_maps(x_q, x_kv, W_qkv, W_out)
    res = run_bass_kernel_spmd(nc, in_maps, list(range(N_CORES)), trace=trace)
    out = combine([r["out"] for r in res.results], b_out)
    return out, res


def kernel(x_q, x_kv, W_qkv, W_out, b_out):
    out, _ = run(x_q, x_kv, W_qkv, W_out, b_out, trace=False)
    return out


# revision 41
# speedup vs baseline: 1.0106x; 1.0106x over previous
"""Multi-head attention (b=2, sq=skv=2048, dim=1024, 16 heads x 64) on 8 TRN2
NeuronCores.

Sharding: 2 heads per core (head-parallel across batch*heads), with the
matching tensor-parallel column slice of W_qkv and row slice of W_out.  Each
core computes a partial output projection over its 128 head-dims; the
all-reduce of the 8 partials (+ bias) happens on the host during unshard.

Per-core kernel (bf16 compute, fp32 PSUM accumulation):
  phase 1: qT/kT/vT = W.T @ x.T   ([128 = 2 heads x 64 dims, tokens]); v is
           additionally PE-transposed to natural [token, dim] layout with a
           ones column appended (denominator trick).
  phase 2: per (batch, q-tile, k-tile): scoresT for both heads ([k-tokens, q])
           in one 2-bank PSUM group; one exp ACTIVATE over the group (scale
           1/8 fused, no max subtraction -- scores range +-10); PV matmuls
           accumulate [v | 1].T @ expT over the 16 k-tiles giving unnormalized
           outT plus the softmax denominator in row 64.  The accumulator is
           copied to SBUF immediately (releasing PSUM); normalization
           (reciprocal + PE outer-product broadcast + multiply) happens off
           the critical path.
  phase 3: partial out = outT.T @ W_out_rows -> bf16 [tokens, 1024].

Emission is orchestrated so the dependency-driven Tile scheduler always has
filler PE work (batch-1 projections, out-projection quarters) inside the
ACT(exp)-bound attention stream, keeping the PE HAM-warm.
"""

import os
import sys

for _p in ("/opt/trn_rl_repo", "/root/.axon_site/_ro/trn_rl_repo"):
    if os.path.isdir(_p) and _p not in sys.path:
        sys.path.append(_p)

import ml_dtypes
import numpy as np

import concourse.bass as bass  # noqa: F401
import concourse.tile as tile
from concourse import bacc, mybir
from concourse.bass_utils import run_bass_kernel_spmd
from concourse.masks import make_identity

B, SQ, SKV, DIM = 2, 2048, 2048, 1024
HEADS, DH = 16, 64
N_CORES = 8
HPC = HEADS // N_CORES  # heads per core = 2
HD = HPC * DH  # 128 head-dim rows per core
TOK = B * SQ  # 4096
KO = DIM // 128  # 8 contraction chunks of 128
SCALE = DH**-0.5

BF16 = mybir.dt.bfloat16
F32 = mybir.dt.float32

PCHUNK = 512  # token chunk in projections (contiguous per-chunk dram layout)
QTILE = 512  # q tile in attention
KTILE = 128  # k tile (scores psum partition dim)
NKT = SKV // KTILE  # 16
NQT = SQ // QTILE  # 4

BF = ml_dtypes.bfloat16
Exp = mybir.ActivationFunctionType.Exp


def build():
    nc = bacc.Bacc(
        "TRN2", target_bir_lowering=False, debug=False, num_devices=N_CORES
    )

    NCH = TOK // PCHUNK
    xqt_d = nc.dram_tensor("xqt", [NCH, 128, KO, PCHUNK], BF16, kind="ExternalInput")
    xkvt_d = nc.dram_tensor("xkvt", [NCH, 128, KO, PCHUNK], BF16, kind="ExternalInput")
    wq_d = nc.dram_tensor("wq", [DIM, HD], BF16, kind="ExternalInput")
    wk_d = nc.dram_tensor("wk", [DIM, HD], BF16, kind="ExternalInput")
    wv_d = nc.dram_tensor("wv", [DIM, HD], BF16, kind="ExternalInput")
    wout_d = nc.dram_tensor("wout", [HD, DIM], BF16, kind="ExternalInput")
    out_d = nc.dram_tensor("out", [TOK, DIM], BF16, kind="ExternalOutput")

    xqt = xqt_d.ap()
    xkvt = xkvt_d.ap()

    with tile.TileContext(nc) as tc:
        with (
            tc.tile_pool(name="persist", bufs=1) as persist,
            tc.tile_pool(name="xin", bufs=6) as xin,
            tc.tile_pool(name="exps", bufs=10) as exps,
            tc.tile_pool(name="ost", bufs=3) as ost,
            tc.tile_pool(name="spsum", bufs=2, space="PSUM") as spsum,
            tc.tile_pool(name="accp", bufs=2, space="PSUM") as accp,
            tc.tile_pool(name="miscp", bufs=2, space="PSUM") as miscp,
            tc.tile_pool(name="drp", bufs=2, space="DRAM") as drp,
        ):
            # --- weights / constants ---
            wq_sb = persist.tile([128, KO, HD], BF16, tag="wq")
            nc.gpsimd.dma_start(wq_sb[:], wq_d.ap().rearrange("(ko p) m -> p ko m", p=128))
            wk_sb = persist.tile([128, KO, HD], BF16, tag="wk")
            nc.gpsimd.dma_start(wk_sb[:], wk_d.ap().rearrange("(ko p) m -> p ko m", p=128))
            wv_sb = persist.tile([128, KO, HD], BF16, tag="wv")
            nc.gpsimd.dma_start(wv_sb[:], wv_d.ap().rearrange("(ko p) m -> p ko m", p=128))
            wout_sb = persist.tile([HD, DIM], BF16, tag="wout")
            nc.gpsimd.dma_start(wout_sb[:], wout_d.ap())

            ident = persist.tile([128, DH], BF16, tag="ident")
            make_identity(nc, ident[0:DH, :])
            make_identity(nc, ident[DH : 2 * DH, :])
            ones_f32 = persist.tile([1, DH], F32, tag="ones")
            nc.vector.memset(ones_f32[:], 1.0)
            # prefetch the exp table set during the head DMAs
            dummy = persist.tile([1, 8], F32, tag="dummy")
            nc.vector.memset(dummy[:], 0.0)
            nc.scalar.activation(dummy[:], dummy[:], Exp)

            qt_sb, kt_sb, vt_sb, vnat, outT, usb = {}, {}, {}, {}, {}, {}
            for b in range(B):
                qt_sb[b] = persist.tile([HD, SQ], BF16, tag=f"qt{b}", name=f"qt{b}")
                kt_sb[b] = persist.tile([HD, SKV], BF16, tag=f"kt{b}", name=f"kt{b}")
                vt_sb[b] = persist.tile([HD, SKV], BF16, tag=f"vt{b}", name=f"vt{b}")
                vnat[b] = persist.tile(
                    [128, HPC, NKT, DH + 1], BF16, tag=f"vn{b}", name=f"vn{b}"
                )
                outT[b] = persist.tile([HD, SQ], BF16, tag=f"ot{b}", name=f"ot{b}")
                # unnormalized outT + denominators, unit index = qt*HPC + h
                usb[b] = persist.tile(
                    [DH + 1, NQT * HPC, QTILE], F32, tag=f"us{b}", name=f"us{b}"
                )
                nc.vector.memset(vnat[b][:, :, :, DH], 1.0)

            def _proj(dst, w_sb, xt, tt):
                for sub in range(PCHUNK // 512):
                    _proj_sub(dst, w_sb, xt, tt, sub)

            def load_chunk(x_ap, tok0, tt, eng=None):
                xt = xin.tile([128, KO, PCHUNK], BF16, tag="x")
                (eng or nc.sync).dma_start(
                    xt[:], x_ap[(tok0 + tt * PCHUNK) // PCHUNK]
                )
                return xt

            def _proj_sub(dst, w_sb, xt, tt, sub):
                ps = miscp.tile([128, 512], F32, tag="m", name="projp")
                for ko in range(KO):
                    nc.tensor.matmul(
                        ps[:],
                        w_sb[:, ko, :],
                        xt[:, ko, sub * 512 : (sub + 1) * 512],
                        start=(ko == 0),
                        stop=(ko == KO - 1),
                    )
                t0 = tt * PCHUNK + sub * 512
                nc.vector.tensor_copy(dst[:, t0 : t0 + 512], ps[:])

            def vnat_group(b, jg):
                """PE-transpose k-tiles 4jg..4jg+3 of vT into natural layout."""
                for h in range(HPC):
                    tp = miscp.tile([128, 4, DH], BF16, tag="m", name="vtp")
                    for i in range(4):
                        j = jg * 4 + i
                        nc.tensor.transpose(
                            tp[:, i, :],
                            vt_sb[b][
                                h * DH : (h + 1) * DH,
                                j * KTILE : (j + 1) * KTILE,
                            ],
                            ident[h * DH : (h + 1) * DH, :],
                        )
                    nc.vector.tensor_copy(
                        vnat[b][:, h, jg * 4 : (jg + 1) * 4, 0:DH], tp[:]
                    )

            F32R = mybir.dt.float32r

            def norm_flush(b, u0, nu):
                """Normalize units u0..u0+nu-1 of usb[b] into outT[b].

                Batches the reciprocal: denominator rows are bounced through
                DRAM to repack [1, nu, QTILE] -> [128, nu*QTILE/128] so the
                DVE reciprocal runs wide, then bounced back and broadcast to
                64 partitions with an f32r PE outer product.
                """
                nel = nu * QTILE
                d1 = drp.tile([1, nu, QTILE], F32, tag="d1", name="d1")
                nc.gpsimd.dma_start(d1[:], usb[b][DH : DH + 1, u0 : u0 + nu, :])
                dpk = ost.tile([128, nel // 128], F32, tag="dp", name="dpk")
                nc.gpsimd.dma_start(
                    dpk[:],
                    d1[:]
                    .rearrange("a b c -> (a b c)")
                    .rearrange("(p f) -> p f", p=128),
                )
                rpk = ost.tile([128, nel // 128], F32, tag="rp", name="rpk")
                nc.vector.reciprocal(rpk[:], dpk[:])
                d2 = drp.tile([1, nu, QTILE], F32, tag="d2", name="d2")
                nc.gpsimd.dma_start(
                    d2[:]
                    .rearrange("a b c -> (a b c)")
                    .rearrange("(p f) -> p f", p=128),
                    rpk[:],
                )
                rst = ost.tile([1, nu, QTILE], F32, tag="rs", name="rst")
                nc.gpsimd.dma_start(rst[:], d2[:])
                for i in range(nu):
                    g = u0 + i
                    qt, h = divmod(g, HPC)
                    bc = miscp.tile([DH, QTILE], F32, tag="m", name="bc")
                    nc.tensor.matmul(
                        bc[:],
                        ones_f32[:].bitcast(F32R),
                        rst[0:1, i, :].bitcast(F32R),
                        start=True,
                        stop=True,
                    )
                    nc.vector.tensor_mul(
                        outT[b][h * DH : (h + 1) * DH, qt * QTILE : (qt + 1) * QTILE],
                        usb[b][0:DH, g, :],
                        bc[:],
                    )

            LOOKAHEAD = 6

            def attention(b, hooks):
                """Flat software-pipelined attention over all (qt, j) steps.

                Scores for step t+2 are emitted before PV of step t, so the
                PE always has score matmuls queued ahead of the exp/PV chain
                and q-tile boundaries pipeline seamlessly.  hooks[qt] is a
                dict keyed (qt, j) of emission callables fired right
                after that step's PV matmuls, spreading filler work finely
                through the ACT-bound stream.
                """
                NT = NQT * NKT
                sps, accs = {}, {}

                def emit_scores(t):
                    qt, j = divmod(t, NKT)
                    q_sl = slice(qt * QTILE, (qt + 1) * QTILE)
                    k_sl = slice(j * KTILE, (j + 1) * KTILE)
                    sp = spsum.tile([128, HPC, QTILE], F32, tag="s", name="sp")
                    sps[t] = sp
                    for h in range(HPC):
                        h_sl = slice(h * DH, (h + 1) * DH)
                        nc.tensor.matmul(
                            sp[:, h, :],
                            kt_sb[b][h_sl, k_sl],
                            qt_sb[b][h_sl, q_sl],
                            start=True,
                            stop=True,
                        )

                def emit_tail(t):
                    qt, j = divmod(t, NKT)
                    sp = sps.pop(t)
                    ex = exps.tile([128, HPC, QTILE], BF16, tag="e", name="ex")
                    nc.scalar.activation(ex[:], sp[:], Exp, scale=SCALE)
                    if j == 0:
                        accs[qt] = [
                            accp.tile([128, QTILE], F32, tag="acc", name="acc")
                            for _ in range(HPC)
                        ]
                    for h in range(HPC):
                        nc.tensor.matmul(
                            accs[qt][h][0 : DH + 1, :],
                            vnat[b][:, h, j, :],
                            ex[:, h, :],
                            start=(j == 0),
                            stop=(j == NKT - 1),
                        )
                    if j == NKT - 1:
                        for h in range(HPC):
                            # free the PSUM accumulator; normalization comes
                            # later in norm_flush
                            nc.vector.tensor_copy(
                                usb[b][:, qt * HPC + h, :],
                                accs[qt][h][0 : DH + 1, :],
                            )
                        del accs[qt]
                    for fn in hooks.get((qt, j), []):
                        fn()

                for t in range(NT + LOOKAHEAD):
                    if t < NT:
                        emit_scores(t)
                    if t >= LOOKAHEAD:
                        emit_tail(t - LOOKAHEAD)

            def outproj(b, tt0, tt1, split_copy=False):
                for tt in range(tt0, tt1):
                    t_sl = slice(tt * 128, (tt + 1) * 128)
                    ob = ost.tile([128, 2, 512], BF16, tag="o")
                    for nt in range(DIM // 512):
                        ps = miscp.tile([128, 512], F32, tag="m", name="projo")
                        nc.tensor.matmul(
                            ps[:],
                            outT[b][:, t_sl],
                            wout_sb[:, nt * 512 : (nt + 1) * 512],
                            start=True,
                            stop=True,
                        )
                        if split_copy and nt % 2 == 0:
                            nc.scalar.copy(ob[:, nt, :], ps[:])
                        else:
                            nc.vector.tensor_copy(ob[:, nt, :], ps[:])
                    nc.gpsimd.dma_start(
                        out_d.ap()[
                            b * SQ + tt * 128 : b * SQ + (tt + 1) * 128, :
                        ].rearrange("t (n c) -> t n c", n=2),
                        ob[:],
                    )

            def qkv_pieces(b):
                """Projection emission steps, 512-token chunks.  K chunks
                first (scores consume them progressively); each x_kv chunk
                is loaded once for both K and V; Q tiles beyond the first
                q-tile come last."""
                xts = {}

                def kv_load_k(tt):
                    xts[tt] = load_chunk(xkvt, b * SKV, tt)
                    _proj(kt_sb[b], wk_sb, xts[tt], tt)

                def v_part(tt):
                    _proj(vt_sb[b], wv_sb, xts.pop(tt), tt)
                    vnat_group(b, tt)

                yield lambda: kv_load_k(0)
                yield lambda: proj_chunk(qt_sb[b], wq_sb, xqt, b * SQ, 0)
                yield lambda: v_part(0)
                for tt in range(1, SQ // PCHUNK):
                    yield lambda tt=tt: kv_load_k(tt)
                    yield lambda tt=tt: v_part(tt)
                for tt in range(1, SQ // PCHUNK):
                    yield lambda tt=tt: proj_chunk(qt_sb[b], wq_sb, xqt, b * SQ, tt)

            def proj_chunk(dst, w_sb, x_ap, tok0, tt):
                _proj(dst, w_sb, load_chunk(x_ap, tok0, tt), tt)

            # --- emission schedule: fine-grained interleave so the scheduler
            # always has dep-free PE work to fill ACT-bound attention gaps ---
            for piece in qkv_pieces(0):
                piece()

            nxt = qkv_pieces(1)

            def emit_next():
                p = next(nxt, None)
                if p is not None:
                    p()

            def emit_n(n):
                def go():
                    for _ in range(n):
                        emit_next()

                return go

            def flush_op(b, qt):
                def go():
                    norm_flush(b, qt * HPC, HPC)
                    outproj(b, qt * 4, qt * 4 + 4)

                return go

            hooks0 = {
                (0, 15): [emit_n(4)],
                (1, 15): [emit_n(4), flush_op(0, 0)],
                (2, 15): [emit_n(4), flush_op(0, 1)],
                (3, 15): [flush_op(0, 2)],
            }
            attention(0, hooks0)

            hooks1 = {
                (0, 0): [lambda: norm_flush(0, 3 * HPC, HPC)],
                (0, 3): [emit_next],
                (0, 8): [lambda: outproj(0, 12, 16), emit_next],
                (0, 13): [emit_next],
                (1, 15): [flush_op(1, 0)],
                (2, 15): [flush_op(1, 1)],
                (3, 15): [flush_op(1, 2)],
            }
            attention(1, hooks1)
            norm_flush(1, 3 * HPC, HPC)
            outproj(1, 12, 16, split_copy=True)

    nc.compile()
    return nc


def make_in_maps(x_q, x_kv, W_qkv, W_out):
    x_q = np.asarray(x_q, dtype=np.float32)
    x_kv = np.asarray(x_kv, dtype=np.float32)
    W_qkv = np.asarray(W_qkv, dtype=np.float32)
    W_out = np.asarray(W_out, dtype=np.float32)

    def chunk_tile(x):
        # [TOK, DIM] -> [n_chunks, 128, KO, PCHUNK] with D = ko*128 + p
        xt = x.reshape(TOK, DIM).T.reshape(KO, 128, TOK // PCHUNK, PCHUNK)
        return np.ascontiguousarray(xt.transpose(2, 1, 0, 3)).astype(BF)

    xqt = chunk_tile(x_q)
    xkvt = chunk_tile(x_kv)

    in_maps = []
    for c in range(N_CORES):
        cs = slice(c * HD, (c + 1) * HD)
        in_maps.append(
            {
                "xqt": xqt,
                "xkvt": xkvt,
                "wq": np.ascontiguousarray(W_qkv[:, cs]).astype(BF),
                "wk": np.ascontiguousarray(W_qkv[:, 1024:][:, cs]).astype(BF),
                "wv": np.ascontiguousarray(W_qkv[:, 2048:][:, cs]).astype(BF),
                "wout": np.ascontiguousarray(W_out[cs, :]).astype(BF),
            }
        )
    return in_maps


def combine(partials, b_out):
    """Sum the 8 per-core partial projections and add the bias."""
    acc = np.zeros((TOK, DIM), dtype=np.float32)
    for p in partials:
        acc += np.asarray(p, dtype=np.float32)
    acc += np.asarray(b_out, dtype=np.float32)
    return acc.reshape(B, SQ, DIM)


_STATE = {}


def _get_nc():
    if "nc" not in _STATE:
        _STATE["nc"] = build()
    return _STATE["nc"]


def run(x_q, x_kv, W_qkv, W_out, b_out, trace=False):
    nc = _get_nc()
    in_maps = make_in_maps(x_q, x_kv, W_qkv, W_out)
    res = run_bass_kernel_spmd(nc, in_maps, list(range(N_CORES)), trace=trace)
    out = combine([r["out"] for r in res.results], b_out)
    return out, res


def kernel(x_q, x_kv, W_qkv, W_out, b_out):
    out, _ = run(x_q, x_kv, W_qkv, W_out, b_out, trace=False)
    return out
